# revision 57
# baseline (speedup 1.0000x reference)
"""MultiHeadAttentionBlock (B=2, S=4096, D=512, H=8) on 8 TRN2 NeuronCores.

Sharding: data-parallel over B (cores 0-3 -> b=0, cores 4-7 -> b=1) x
tensor-parallel over heads (2 heads per core, column-parallel Wq/Wk/Wv,
row-parallel Wo). Each core computes a partial output (4096, 512); the host
sums the 4 partials per batch and adds the folded bias (Wo @ bv + bo).

Device-side math per core (heads A, B):
  QT/KT = (Wq/Wk slice) @ x^T + b            -> (128=2*64 dims, 4096) fp16
  V     = x^T-chunks^T @ Wv slice            -> V_ext (t, [v|1]) fp16
  ST    = K_h Q_h^T per 128-t-block          -> PSUM (t=128, s)
  E     = exp(ST/8 + maskbias)               -> fp16  (no max-subtraction:
          scores are N(0,1)-scale, exp is safe; identical through softmax)
  ctx^T,den = [V|1]^T E                      -> PSUM (65, s) accumulated over t
  ctxn  = ctx^T * (1/den broadcast)          -> fp16
  out   = ctxn_A^T woA + ctxn_B^T woB        -> (4096, 512) fp32 partial
"""

import os
import subprocess
import sys
import tempfile

import numpy as np

B, S, D, H, DK = 2, 4096, 512, 8, 64
N_CORES = 8
HEADS_PER_CORE = 2
HD = HEADS_PER_CORE * DK  # 128 head-dims per core
TB = S // 128  # 32 t-blocks
SO = 8  # s_outer chunks
SOW = S // SO  # 512 wide


def _build_nc():
    sys.path.insert(0, "/opt/trn_rl_repo")
    from contextlib import ExitStack

    import concourse.tile as tile
    from concourse import bacc, mybir
    from concourse.tile import add_dep_helper

    F32 = mybir.dt.float32
    F16 = mybir.dt.float16
    EXP = mybir.ActivationFunctionType.Exp

    nc = bacc.Bacc("TRN2", target_bir_lowering=False, debug=False)

    xq_d = nc.dram_tensor("xq", [D, S], F16, kind="ExternalInput").ap()
    xk_d = nc.dram_tensor("xk", [D, S], F16, kind="ExternalInput").ap()
    xv_d = nc.dram_tensor("xv", [D, S], F16, kind="ExternalInput").ap()
    wq_d = nc.dram_tensor("wq", [D, HD], F16, kind="ExternalInput").ap()
    wk_d = nc.dram_tensor("wk", [D, HD], F16, kind="ExternalInput").ap()
    wv_d = nc.dram_tensor("wv", [D, HD], F16, kind="ExternalInput").ap()
    wo_d = nc.dram_tensor("wo", [HD, D], F16, kind="ExternalInput").ap()
    bq_d = nc.dram_tensor("bq", [HD, 1], F32, kind="ExternalInput").ap()
    bk_d = nc.dram_tensor("bk", [HD, 1], F32, kind="ExternalInput").ap()
    mb_d = nc.dram_tensor("maskbias", [128, TB], F32, kind="ExternalInput").ap()
    o_d = nc.dram_tensor("o", [S, D], F32, kind="ExternalOutput").ap()
    rden_d = nc.dram_tensor("rden", [2 * SO, SOW], F32).ap()  # recip-den bounce

    xq_r = xq_d.rearrange("(c p) s -> p c s", p=128)
    xk_r = xk_d.rearrange("(c p) s -> p c s", p=128)
    xv_r = xv_d.rearrange("(c p) s -> p c s", p=128)
    wq_r = wq_d.rearrange("(c p) j -> p c j", p=128)
    wk_r = wk_d.rearrange("(c p) j -> p c j", p=128)
    wv_r = wv_d.rearrange("(c p) j -> p c j", p=128)

    with tile.TileContext(nc) as tc, ExitStack() as ctx:
        const = ctx.enter_context(tc.tile_pool(name="const", bufs=1))
        persist = ctx.enter_context(tc.tile_pool(name="persist", bufs=1))

        wq_t = const.tile([128, 4, HD], F16)
        wk_t = const.tile([128, 4, HD], F16)
        wv_t = const.tile([128, 4, HD], F16)
        wo_t = const.tile([HD, D], F16)
        bq_t = const.tile([HD, 1], F32)
        bk_t = const.tile([HD, 1], F32)
        mb_t = const.tile([128, TB], F32)
        nc.sync.dma_start(wq_t[:], wq_r[:])
        nc.sync.dma_start(wk_t[:], wk_r[:])
        nc.sync.dma_start(wv_t[:], wv_r[:])
        nc.sync.dma_start(wo_t[:], wo_d[:])
        nc.sync.dma_start(bq_t[:], bq_d[:])
        nc.sync.dma_start(bk_t[:], bk_d[:])
        nc.sync.dma_start(mb_t[:], mb_d[:])

        qt = persist.tile([128, S], F16)  # rows 0-63 head A, 64-127 head B
        kt = persist.tile([128, S], F16)
        vea = persist.tile([128, TB, DK + 1], F16)  # [v | ones]
        veb = persist.tile([128, TB, DK + 1], F16)
        # combined normalized ctx^T: rows 0-63 head A (written by DVE), rows
        # 64-127 head B (partition-shifted in by DMA from ctxb per s_outer)
        ctxc = persist.tile([128, S], F16)
        ctxb = persist.tile([DK, S], F16)
        nc.vector.memset(vea[:, :, DK : DK + 1], 1.0)
        nc.vector.memset(veb[:, :, DK : DK + 1], 1.0)

        # ---------------- P1: upfront loads + K projection ----------------
        # K is fully projected upfront (QK needs every t-block); Q/V beyond
        # the first chunk are projected inside the attention loop, just in
        # time, using one spare PSUM bank.
        inp = ctx.enter_context(tc.tile_pool(name="inp", bufs=1))
        pps = ctx.enter_context(tc.tile_pool(name="pps", bufs=2, space="PSUM"))

        # prefetch the exp table set while ACT is otherwise idle
        warm_in = const.tile([1, 2], F32)
        warm_out = const.tile([1, 2], F32)
        nc.vector.memset(warm_in[:], 0.0)
        nc.scalar.activation(warm_out[:], warm_in[:], EXP)

        xq_tiles, xv_tiles, xk_tiles = [], [], []
        for sc in range(8):
            sl = slice(sc * 512, (sc + 1) * 512)
            xk_t = inp.tile([128, 4, 512], F16, tag=f"xk{sc}")
            nc.gpsimd.dma_start(xk_t[:], xk_r[:, :, sl])
            xk_tiles.append(xk_t)
        for sc in range(8):
            sl = slice(sc * 512, (sc + 1) * 512)
            xq_t = inp.tile([128, 4, 512], F16, tag=f"xq{sc}")
            xv_t = inp.tile([128, 4, 512], F16, tag=f"xv{sc}")
            nc.sync.dma_start(xq_t[:], xq_r[:, :, sl])
            nc.gpsimd.dma_start(xv_t[:], xv_r[:, :, sl])
            xq_tiles.append(xq_t)
            xv_tiles.append(xv_t)

        k_halves = {}

        def k_proj_half(sc, half):
            sl = slice(sc * 512, (sc + 1) * 512)
            if half == 0:
                k_halves[sc] = pps.tile([128, 512], F32, tag="pps", name="kps")
            kps = k_halves[sc]
            for c in (2 * half, 2 * half + 1):
                nc.tensor.matmul(
                    kps[:], wk_t[:, c, :], xk_tiles[sc][:, c, :],
                    start=(c == 0), stop=(c == 3),
                )
            if half == 1:
                nc.vector.tensor_scalar_add(kt[:, sl], kps[:], bk_t[:])

        def k_proj(sc):
            k_proj_half(sc, 0)
            k_proj_half(sc, 1)

        def q_proj(sc):
            sl = slice(sc * 512, (sc + 1) * 512)
            qps = pps.tile([128, 512], F32, tag="pps")
            for c in range(4):
                nc.tensor.matmul(
                    qps[:], wq_t[:, c, :], xq_tiles[sc][:, c, :],
                    start=(c == 0), stop=(c == 3),
                )
            nc.vector.tensor_scalar_add(qt[:, sl], qps[:], bq_t[:])

        def v_proj(sc, tl):
            tb = sc * 4 + tl
            vps_full = pps.tile([128, 512], F32, tag="pps", name="vps_full")
            vps = vps_full[:, 0:HD]
            for c in range(4):
                nc.tensor.matmul(
                    vps[:],
                    xv_tiles[sc][:, c, tl * 128 : (tl + 1) * 128],
                    wv_t[:, c, :],
                    start=(c == 0), stop=(c == 3),
                )
            nc.vector.tensor_copy(vea[:, tb, 0:DK], vps[:, 0:DK])
            nc.vector.tensor_copy(veb[:, tb, 0:DK], vps[:, DK:HD])

        osb = ctx.enter_context(tc.tile_pool(name="osb", bufs=4))
        oeng = [nc.sync, nc.gpsimd]

        def out_proj(sb):
            ssl = slice(sb * 128, (sb + 1) * 128)
            ops = pps.tile([128, 512], F32, tag="pps", name="ops")
            nc.tensor.matmul(ops[:], ctxc[:, ssl], wo_t[:], start=True, stop=True)
            o_sb = osb.tile([128, D], F32, tag="osb")
            nc.vector.tensor_copy(o_sb[:], ops[:])
            oeng[sb % 2].dma_start(o_d[ssl, :], o_sb[:])

        k_proj(0)
        q_proj(0)

        # leftover work scheduled by global iteration index (33 per s_outer):
        # K chunk sc before QK needs it at iter 4*sc, V chunk for t-block t
        # before PV reads it at iter t+1, Q chunk sc long before s_outer sc
        leftovers = {
            0: [(v_proj, 0, 0), (v_proj, 0, 1)],
            1: [(v_proj, 0, 2)],
            2: [(v_proj, 0, 3)],
        }
        for sc in range(1, 8):
            leftovers.setdefault(2 * sc - 1, []).append((k_proj_half, sc, 0))
            leftovers.setdefault(2 * sc, []).append((k_proj_half, sc, 1))
            for tl in range(4):
                leftovers.setdefault(4 * (sc - 1) + tl + 2, []).append(
                    (v_proj, sc, tl)
                )
            leftovers.setdefault(26 + 3 * sc, []).append((q_proj, sc, None))

        # ---------------- P2: attention ----------------
        with ExitStack() as p2:
            stp = p2.enter_context(tc.tile_pool(name="stp", bufs=2, space="PSUM"))
            cxp = p2.enter_context(tc.tile_pool(name="cxp", bufs=2, space="PSUM"))
            ep = p2.enter_context(tc.tile_pool(name="ep", bufs=4))
            dnp = p2.enter_context(tc.tile_pool(name="dnp", bufs=2))

            for so in range(SO):
                s0 = so * SOW
                ca_ps = cxp.tile([DK + 1, SOW], F32, tag="cx")
                cb_ps = cxp.tile([DK + 1, SOW], F32, tag="cx")
                e_prev = None
                for tb in range(TB + 1):
                    if tb < TB:
                        tsl = slice(tb * 128, (tb + 1) * 128)
                        # single ST tile [A half | B half], double-buffered:
                        # QK(tb+1) writes the other buffer, so the WAR wait is
                        # against ACT(tb-1) — two iterations of lookahead
                        st = stp.tile([128, 2 * SOW], F32, tag="st")
                        e = ep.tile([128, 2 * SOW], F16, tag="e")
                        prev_mm = None
                        for h in range(2):
                            hsl = slice(h * 64, (h + 1) * 64)
                            mm = nc.tensor.matmul(
                                st[:, h * SOW : (h + 1) * SOW],
                                kt[hsl, tsl],
                                qt[hsl, s0 : s0 + SOW],
                                start=True, stop=True,
                                tile_position=(h * 64, 0),
                            )
                            if prev_mm is not None:
                                add_dep_helper(
                                    mm.ins, prev_mm.ins,
                                    sync=False, reason="qk pair order",
                                )
                            prev_mm = mm
                        nc.scalar.activation(
                            e[:],
                            st[:],
                            EXP,
                            bias=mb_t[:, tb : tb + 1],
                            scale=0.125,
                        )
                    if tb >= 1:
                        ptb = tb - 1
                        nc.tensor.matmul(
                            ca_ps[:],
                            vea[:, ptb, :],
                            e_prev[:, 0:SOW],
                            start=(ptb == 0), stop=(ptb == TB - 1),
                        )
                        nc.tensor.matmul(
                            cb_ps[:],
                            veb[:, ptb, :],
                            e_prev[:, SOW : 2 * SOW],
                            start=(ptb == 0), stop=(ptb == TB - 1),
                        )
                    e_prev = e
                    it = so * (TB + 1) + tb
                    for fn, a1, a2 in leftovers.pop(it, []):
                        if a2 is None:
                            fn(a1)
                        else:
                            fn(a1, a2)

                # drain: fast PSUM->SBUF copies first (frees the ctx banks for
                # the next s_outer), then reciprocal on a partition-broadcast
                # copy (full-width DVE, not 1-lane) and normalize into fp16
                for h, (cps, ctx_t) in enumerate(
                    ((ca_ps, ctxc[0:DK]), (cb_ps, ctxb))
                ):
                    cxf = dnp.tile([DK, SOW], F32, tag="cxf")
                    den = dnp.tile([1, SOW], F32, tag="den")
                    nc.vector.tensor_copy(den[:], cps[DK : DK + 1, :])
                    nc.vector.tensor_copy(cxf[:], cps[0:DK, :])
                    row = 2 * so + h
                    nc.sync.dma_start(rden_d[row : row + 1, :], den[:])
                    rbc = dnp.tile([DK, SOW], F32, tag="rbc")
                    nc.sync.dma_start(
                        rbc[:], rden_d[row : row + 1, :].to_broadcast((DK, SOW))
                    )
                    nc.vector.reciprocal(rbc[:], rbc[:])
                    nc.vector.tensor_tensor(
                        ctx_t[:, s0 : s0 + SOW], cxf[:], rbc[:],
                        mybir.AluOpType.mult,
                    )
                # shift head B's ctx rows into partitions 64-127 of the
                # combined tile (DMA can cross partitions; DVE cannot)
                nc.gpsimd.dma_start(
                    ctxc[DK:128, s0 : s0 + SOW], ctxb[:, s0 : s0 + SOW]
                )

        # ---------------- P3: output projection ----------------
        for sb in range(S // 128):
            out_proj(sb)

    nc.compile()
    return nc


def _prep_in_maps(inputs):
    q = inputs["q"]
    k = inputs["k"]
    v = inputs["v"]
    mask = inputs["mask"]
    Wq, bq = inputs["Wq"], inputs["bq"]
    Wk, bk = inputs["Wk"], inputs["bk"]
    Wv, bv = inputs["Wv"], inputs["bv"]
    Wo = inputs["Wo"]

    in_maps = []
    for c in range(N_CORES):
        b = c // 4
        h0 = (c % 4) * HEADS_PER_CORE
        rows = slice(h0 * DK, h0 * DK + HD)
        xq = np.ascontiguousarray(q[b].T.astype(np.float16))
        xk = np.ascontiguousarray(k[b].T.astype(np.float16))
        xv = np.ascontiguousarray(v[b].T.astype(np.float16))
        wq = np.ascontiguousarray(Wq[rows, :].T.astype(np.float16))
        wk = np.ascontiguousarray(Wk[rows, :].T.astype(np.float16))
        wv = np.ascontiguousarray(Wv[rows, :].T.astype(np.float16))
        wo = np.ascontiguousarray(Wo[:, rows].T.astype(np.float16))
        bq_s = np.ascontiguousarray(bq[rows].reshape(HD, 1).astype(np.float32))
        bk_s = np.ascontiguousarray(bk[rows].reshape(HD, 1).astype(np.float32))
        m = mask[b, 0, 0].astype(np.float32)  # (S,)
        maskbias = np.ascontiguousarray(
            ((m - 1.0) * 1.25e8).reshape(TB, 128).T.astype(np.float32)
        )
        in_maps.append(
            {
                "xq": xq, "xk": xk, "xv": xv,
                "wq": wq, "wk": wk, "wv": wv,
                "wo": wo,
                "bq": bq_s, "bk": bk_s,
                "maskbias": maskbias,
            }
        )
    return in_maps


def _assemble(inputs, results):
    Wo, bv, bo = inputs["Wo"], inputs["bv"], inputs["bo"]
    fold = (Wo.astype(np.float32) @ bv.astype(np.float32)) + bo.astype(np.float32)
    out = np.zeros((B, S, D), dtype=np.float32)
    for c in range(N_CORES):
        out[c // 4] += results[c]["o"]
    out += fold[None, None, :]
    return out


def _run(inputs, trace=False):
    sys.path.insert(0, "/opt/trn_rl_repo")
    from concourse.bass_utils import run_bass_kernel_spmd

    nc = _build_nc()
    in_maps = _prep_in_maps(inputs)
    res = run_bass_kernel_spmd(nc, in_maps, list(range(N_CORES)), trace=trace)
    return _assemble(inputs, res.results), res.exec_time_ns


def kernel(**inputs):
    """Full unsharded inputs -> full (2, 4096, 512) fp32 output."""
    # jax must see the axon TRN backend; if the host process pinned
    # JAX_PLATFORMS (e.g. to cpu) or already initialized jax, run the device
    # part in a clean subprocess.
    need_sub = False
    if "jax" in sys.modules:
        try:
            import jax

            need_sub = not any(d.platform == "axon" for d in jax.devices())
        except Exception:
            need_sub = True
    if os.environ.get("JAX_PLATFORMS") not in (None, "") and "axon" not in str(
        os.environ.get("JAX_PLATFORMS")
    ):
        need_sub = True

    if not need_sub:
        out, _ = _run(inputs)
        return out

    with tempfile.TemporaryDirectory() as td:
        in_path = os.path.join(td, "in.npz")
        out_path = os.path.join(td, "out.npy")
        np.savez(in_path, **inputs)
        env = dict(os.environ)
        env.pop("JAX_PLATFORMS", None)
        subprocess.run(
            [sys.executable, os.path.abspath(__file__), "--worker", in_path, out_path],
            check=True,
            env=env,
        )
        return np.load(out_path)


if __name__ == "__main__":
    if len(sys.argv) >= 4 and sys.argv[1] == "--worker":
        data = dict(np.load(sys.argv[2]))
        out, _ = _run(data)
        np.save(sys.argv[3], out)


# revision 58
# speedup vs baseline: 1.0628x; 1.0628x over previous
"""MultiHeadAttentionBlock (B=2, S=4096, D=512, H=8) on 8 TRN2 NeuronCores.

Sharding: data-parallel over B (cores 0-3 -> b=0, cores 4-7 -> b=1) x
tensor-parallel over heads (2 heads per core, column-parallel Wq/Wk/Wv,
row-parallel Wo). Each core computes a partial output (4096, 512); the host
sums the 4 partials per batch and adds the folded bias (Wo @ bv + bo).

Device-side math per core (heads A, B):
  QT/KT = (Wq/Wk slice) @ x^T + b            -> (128=2*64 dims, 4096) fp16
  V     = x^T-chunks^T @ Wv slice            -> V_ext (t, [v|1]) fp16
  ST    = K_h Q_h^T per 128-t-block          -> PSUM (t=128, s)
  E     = exp(ST/8 + maskbias)               -> fp16  (no max-subtraction:
          scores are N(0,1)-scale, exp is safe; identical through softmax)
  ctx^T,den = [V|1]^T E                      -> PSUM (65, s) accumulated over t
  ctxn  = ctx^T * (1/den broadcast)          -> fp16
  out   = ctxn_A^T woA + ctxn_B^T woB        -> (4096, 512) fp32 partial
"""

import os
import subprocess
import sys
import tempfile

import numpy as np

B, S, D, H, DK = 2, 4096, 512, 8, 64
N_CORES = 8
HEADS_PER_CORE = 2
HD = HEADS_PER_CORE * DK  # 128 head-dims per core
TB = S // 128  # 32 t-blocks
SO = 8  # s_outer chunks
SOW = S // SO  # 512 wide


def _build_nc():
    sys.path.insert(0, "/opt/trn_rl_repo")
    from contextlib import ExitStack

    import concourse.tile as tile
    from concourse import bacc, mybir
    from concourse.tile import add_dep_helper

    F32 = mybir.dt.float32
    F16 = mybir.dt.float16
    EXP = mybir.ActivationFunctionType.Exp

    nc = bacc.Bacc("TRN2", target_bir_lowering=False, debug=False)

    xq_d = nc.dram_tensor("xq", [D, S], F16, kind="ExternalInput").ap()
    xk_d = nc.dram_tensor("xk", [D, S], F16, kind="ExternalInput").ap()
    xv_d = nc.dram_tensor("xv", [D, S], F16, kind="ExternalInput").ap()
    wq_d = nc.dram_tensor("wq", [D, HD], F16, kind="ExternalInput").ap()
    wk_d = nc.dram_tensor("wk", [D, HD], F16, kind="ExternalInput").ap()
    wv_d = nc.dram_tensor("wv", [D, HD], F16, kind="ExternalInput").ap()
    wo_d = nc.dram_tensor("wo", [HD, D], F16, kind="ExternalInput").ap()
    bq_d = nc.dram_tensor("bq", [HD, 1], F32, kind="ExternalInput").ap()
    bk_d = nc.dram_tensor("bk", [HD, 1], F32, kind="ExternalInput").ap()
    mb_d = nc.dram_tensor("maskbias", [128, TB], F32, kind="ExternalInput").ap()
    o_d = nc.dram_tensor("o", [S, D], F32, kind="ExternalOutput").ap()
    rden_d = nc.dram_tensor("rden", [2 * SO, SOW], F32).ap()  # recip-den bounce

    xq_r = xq_d.rearrange("(c p) s -> p c s", p=128)
    xk_r = xk_d.rearrange("(c p) s -> p c s", p=128)
    xv_r = xv_d.rearrange("(c p) s -> p c s", p=128)
    wq_r = wq_d.rearrange("(c p) j -> p c j", p=128)
    wk_r = wk_d.rearrange("(c p) j -> p c j", p=128)
    wv_r = wv_d.rearrange("(c p) j -> p c j", p=128)

    with tile.TileContext(nc) as tc, ExitStack() as ctx:
        const = ctx.enter_context(tc.tile_pool(name="const", bufs=1))
        persist = ctx.enter_context(tc.tile_pool(name="persist", bufs=1))

        wq_t = const.tile([128, 4, HD], F16)
        wk_t = const.tile([128, 4, HD], F16)
        wv_t = const.tile([128, 4, HD], F16)
        wo_t = const.tile([HD, D], F16)
        bq_t = const.tile([HD, 1], F32)
        bk_t = const.tile([HD, 1], F32)
        mb_t = const.tile([128, TB], F32)
        nc.sync.dma_start(wq_t[:], wq_r[:])
        nc.sync.dma_start(wk_t[:], wk_r[:])
        nc.sync.dma_start(wv_t[:], wv_r[:])
        nc.sync.dma_start(wo_t[:], wo_d[:])
        nc.sync.dma_start(bq_t[:], bq_d[:])
        nc.sync.dma_start(bk_t[:], bk_d[:])
        nc.sync.dma_start(mb_t[:], mb_d[:])

        qt = persist.tile([128, S], F16)  # rows 0-63 head A, 64-127 head B
        kt = persist.tile([128, S], F16)
        vea = persist.tile([128, TB, DK + 1], F16)  # [v | ones]
        veb = persist.tile([128, TB, DK + 1], F16)
        # combined normalized ctx^T: rows 0-63 head A (written by DVE), rows
        # 64-127 head B (partition-shifted in by DMA from ctxb per s_outer)
        ctxc = persist.tile([128, S], F16)
        ctxb = persist.tile([DK, S], F16)
        nc.vector.memset(vea[:, :, DK : DK + 1], 1.0)
        nc.vector.memset(veb[:, :, DK : DK + 1], 1.0)

        # ---------------- P1: upfront loads + K projection ----------------
        # K is fully projected upfront (QK needs every t-block); Q/V beyond
        # the first chunk are projected inside the attention loop, just in
        # time, using one spare PSUM bank.
        inp = ctx.enter_context(tc.tile_pool(name="inp", bufs=1))
        pps = ctx.enter_context(tc.tile_pool(name="pps", bufs=2, space="PSUM"))

        # prefetch the exp table set while ACT is otherwise idle
        warm_in = const.tile([1, 2], F32)
        warm_out = const.tile([1, 2], F32)
        nc.vector.memset(warm_in[:], 0.0)
        nc.scalar.activation(warm_out[:], warm_in[:], EXP)

        xq_tiles, xv_tiles, xk_tiles = [], [], []
        for sc in range(8):
            sl = slice(sc * 512, (sc + 1) * 512)
            xk_t = inp.tile([128, 4, 512], F16, tag=f"xk{sc}")
            nc.gpsimd.dma_start(xk_t[:], xk_r[:, :, sl])
            xk_tiles.append(xk_t)
        for sc in range(8):
            sl = slice(sc * 512, (sc + 1) * 512)
            xq_t = inp.tile([128, 4, 512], F16, tag=f"xq{sc}")
            xv_t = inp.tile([128, 4, 512], F16, tag=f"xv{sc}")
            nc.sync.dma_start(xq_t[:], xq_r[:, :, sl])
            nc.gpsimd.dma_start(xv_t[:], xv_r[:, :, sl])
            xq_tiles.append(xq_t)
            xv_tiles.append(xv_t)

        k_halves = {}

        def k_proj_half(sc, half):
            sl = slice(sc * 512, (sc + 1) * 512)
            if half == 0:
                k_halves[sc] = pps.tile([128, 512], F32, tag="pps", name="kps")
            kps = k_halves[sc]
            for c in (2 * half, 2 * half + 1):
                nc.tensor.matmul(
                    kps[:], wk_t[:, c, :], xk_tiles[sc][:, c, :],
                    start=(c == 0), stop=(c == 3),
                )
            if half == 1:
                nc.vector.tensor_scalar_add(kt[:, sl], kps[:], bk_t[:])

        def k_proj(sc):
            k_proj_half(sc, 0)
            k_proj_half(sc, 1)

        def q_proj(sc):
            sl = slice(sc * 512, (sc + 1) * 512)
            qps = pps.tile([128, 512], F32, tag="pps")
            for c in range(4):
                nc.tensor.matmul(
                    qps[:], wq_t[:, c, :], xq_tiles[sc][:, c, :],
                    start=(c == 0), stop=(c == 3),
                )
            nc.vector.tensor_scalar_add(qt[:, sl], qps[:], bq_t[:])

        def v_proj(sc, tl):
            tb = sc * 4 + tl
            vps_full = pps.tile([128, 512], F32, tag="pps", name="vps_full")
            vps = vps_full[:, 0:HD]
            for c in range(4):
                nc.tensor.matmul(
                    vps[:],
                    xv_tiles[sc][:, c, tl * 128 : (tl + 1) * 128],
                    wv_t[:, c, :],
                    start=(c == 0), stop=(c == 3),
                )
            nc.vector.tensor_copy(vea[:, tb, 0:DK], vps[:, 0:DK])
            nc.vector.tensor_copy(veb[:, tb, 0:DK], vps[:, DK:HD])

        k_proj(0)
        q_proj(0)
        v_proj(0, 0)
        v_proj(0, 1)

        # leftover work scheduled by global iteration index (33 per s_outer):
        # K chunk sc before QK needs it at iter 4*sc, V chunk for t-block t
        # before PV reads it at iter t+1, Q chunk sc long before s_outer sc
        leftovers = {1: [(v_proj, 0, 2)], 2: [(v_proj, 0, 3)]}
        for sc in range(1, 8):
            leftovers.setdefault(2 * sc - 1, []).append((k_proj_half, sc, 0))
            leftovers.setdefault(2 * sc, []).append((k_proj_half, sc, 1))
            for tl in range(4):
                leftovers.setdefault(4 * (sc - 1) + tl + 2, []).append(
                    (v_proj, sc, tl)
                )
            leftovers.setdefault(26 + 3 * sc, []).append((q_proj, sc, None))

        # ---------------- P2: attention ----------------
        with ExitStack() as p2:
            stp = p2.enter_context(tc.tile_pool(name="stp", bufs=2, space="PSUM"))
            cxp = p2.enter_context(tc.tile_pool(name="cxp", bufs=2, space="PSUM"))
            ep = p2.enter_context(tc.tile_pool(name="ep", bufs=4))
            dnp = p2.enter_context(tc.tile_pool(name="dnp", bufs=2))

            for so in range(SO):
                s0 = so * SOW
                ca_ps = cxp.tile([DK + 1, SOW], F32, tag="cx")
                cb_ps = cxp.tile([DK + 1, SOW], F32, tag="cx")
                e_prev = None
                for tb in range(TB + 1):
                    if tb < TB:
                        tsl = slice(tb * 128, (tb + 1) * 128)
                        # single ST tile [A half | B half], double-buffered:
                        # QK(tb+1) writes the other buffer, so the WAR wait is
                        # against ACT(tb-1) — two iterations of lookahead
                        st = stp.tile([128, 2 * SOW], F32, tag="st")
                        e = ep.tile([128, 2 * SOW], F16, tag="e")
                        prev_mm = None
                        for h in range(2):
                            hsl = slice(h * 64, (h + 1) * 64)
                            mm = nc.tensor.matmul(
                                st[:, h * SOW : (h + 1) * SOW],
                                kt[hsl, tsl],
                                qt[hsl, s0 : s0 + SOW],
                                start=True, stop=True,
                                tile_position=(h * 64, 0),
                            )
                            if prev_mm is not None:
                                add_dep_helper(
                                    mm.ins, prev_mm.ins,
                                    sync=False, reason="qk pair order",
                                )
                            prev_mm = mm
                        nc.scalar.activation(
                            e[:],
                            st[:],
                            EXP,
                            bias=mb_t[:, tb : tb + 1],
                            scale=0.125,
                        )
                    if tb >= 1:
                        ptb = tb - 1
                        nc.tensor.matmul(
                            ca_ps[:],
                            vea[:, ptb, :],
                            e_prev[:, 0:SOW],
                            start=(ptb == 0), stop=(ptb == TB - 1),
                        )
                        nc.tensor.matmul(
                            cb_ps[:],
                            veb[:, ptb, :],
                            e_prev[:, SOW : 2 * SOW],
                            start=(ptb == 0), stop=(ptb == TB - 1),
                        )
                    e_prev = e
                    it = so * (TB + 1) + tb
                    for fn, a1, a2 in leftovers.pop(it, []):
                        if a2 is None:
                            fn(a1)
                        else:
                            fn(a1, a2)

                # drain: fast PSUM->SBUF copies first (frees the ctx banks for
                # the next s_outer), then reciprocal on a partition-broadcast
                # copy (full-width DVE, not 1-lane) and normalize into fp16
                for h, (cps, ctx_t) in enumerate(
                    ((ca_ps, ctxc[0:DK]), (cb_ps, ctxb))
                ):
                    cxf = dnp.tile([DK, SOW], F32, tag="cxf")
                    den = dnp.tile([1, SOW], F32, tag="den")
                    nc.vector.tensor_copy(den[:], cps[DK : DK + 1, :])
                    nc.vector.tensor_copy(cxf[:], cps[0:DK, :])
                    row = 2 * so + h
                    nc.sync.dma_start(rden_d[row : row + 1, :], den[:])
                    rbc = dnp.tile([DK, SOW], F32, tag="rbc")
                    nc.sync.dma_start(
                        rbc[:], rden_d[row : row + 1, :].to_broadcast((DK, SOW))
                    )
                    nc.vector.reciprocal(rbc[:], rbc[:])
                    nc.vector.tensor_tensor(
                        ctx_t[:, s0 : s0 + SOW], cxf[:], rbc[:],
                        mybir.AluOpType.mult,
                    )
                # shift head B's ctx rows into partitions 64-127 of the
                # combined tile (DMA can cross partitions; DVE cannot)
                nc.gpsimd.dma_start(
                    ctxc[DK:128, s0 : s0 + SOW], ctxb[:, s0 : s0 + SOW]
                )

        # ---------------- P3: output projection ----------------
        with ExitStack() as p3:
            ops_pool = p3.enter_context(tc.tile_pool(name="ops", bufs=6, space="PSUM"))
            osb = p3.enter_context(tc.tile_pool(name="osb", bufs=4))
            oeng = [nc.sync, nc.gpsimd]
            for sb in range(S // 128):
                ssl = slice(sb * 128, (sb + 1) * 128)
                ops = ops_pool.tile([128, D], F32, tag="ops")
                nc.tensor.matmul(ops[:], ctxc[:, ssl], wo_t[:], start=True, stop=True)
                o_sb = osb.tile([128, D], F32, tag="osb")
                if sb % 2 == 0:
                    nc.vector.tensor_copy(o_sb[:], ops[:])
                else:
                    nc.scalar.copy(o_sb[:], ops[:])
                oeng[sb % 2].dma_start(o_d[ssl, :], o_sb[:])

    nc.compile()
    return nc


def _prep_in_maps(inputs):
    q = inputs["q"]
    k = inputs["k"]
    v = inputs["v"]
    mask = inputs["mask"]
    Wq, bq = inputs["Wq"], inputs["bq"]
    Wk, bk = inputs["Wk"], inputs["bk"]
    Wv, bv = inputs["Wv"], inputs["bv"]
    Wo = inputs["Wo"]

    in_maps = []
    for c in range(N_CORES):
        b = c // 4
        h0 = (c % 4) * HEADS_PER_CORE
        rows = slice(h0 * DK, h0 * DK + HD)
        xq = np.ascontiguousarray(q[b].T.astype(np.float16))
        xk = np.ascontiguousarray(k[b].T.astype(np.float16))
        xv = np.ascontiguousarray(v[b].T.astype(np.float16))
        wq = np.ascontiguousarray(Wq[rows, :].T.astype(np.float16))
        wk = np.ascontiguousarray(Wk[rows, :].T.astype(np.float16))
        wv = np.ascontiguousarray(Wv[rows, :].T.astype(np.float16))
        wo = np.ascontiguousarray(Wo[:, rows].T.astype(np.float16))
        bq_s = np.ascontiguousarray(bq[rows].reshape(HD, 1).astype(np.float32))
        bk_s = np.ascontiguousarray(bk[rows].reshape(HD, 1).astype(np.float32))
        m = mask[b, 0, 0].astype(np.float32)  # (S,)
        maskbias = np.ascontiguousarray(
            ((m - 1.0) * 1.25e8).reshape(TB, 128).T.astype(np.float32)
        )
        in_maps.append(
            {
                "xq": xq, "xk": xk, "xv": xv,
                "wq": wq, "wk": wk, "wv": wv,
                "wo": wo,
                "bq": bq_s, "bk": bk_s,
                "maskbias": maskbias,
            }
        )
    return in_maps


def _assemble(inputs, results):
    Wo, bv, bo = inputs["Wo"], inputs["bv"], inputs["bo"]
    fold = (Wo.astype(np.float32) @ bv.astype(np.float32)) + bo.astype(np.float32)
    out = np.zeros((B, S, D), dtype=np.float32)
    for c in range(N_CORES):
        out[c // 4] += results[c]["o"]
    out += fold[None, None, :]
    return out


def _run(inputs, trace=False):
    sys.path.insert(0, "/opt/trn_rl_repo")
    from concourse.bass_utils import run_bass_kernel_spmd

    nc = _build_nc()
    in_maps = _prep_in_maps(inputs)
    res = run_bass_kernel_spmd(nc, in_maps, list(range(N_CORES)), trace=trace)
    return _assemble(inputs, res.results), res.exec_time_ns


def kernel(**inputs):
    """Full unsharded inputs -> full (2, 4096, 512) fp32 output."""
    # jax must see the axon TRN backend; if the host process pinned
    # JAX_PLATFORMS (e.g. to cpu) or already initialized jax, run the device
    # part in a clean subprocess.
    need_sub = False
    if "jax" in sys.modules:
        try:
            import jax

            need_sub = not any(d.platform == "axon" for d in jax.devices())
        except Exception:
            need_sub = True
    if os.environ.get("JAX_PLATFORMS") not in (None, "") and "axon" not in str(
        os.environ.get("JAX_PLATFORMS")
    ):
        need_sub = True

    if not need_sub:
        out, _ = _run(inputs)
        return out

    with tempfile.TemporaryDirectory() as td:
        in_path = os.path.join(td, "in.npz")
        out_path = os.path.join(td, "out.npy")
        np.savez(in_path, **inputs)
        env = dict(os.environ)
        env.pop("JAX_PLATFORMS", None)
        subprocess.run(
            [sys.executable, os.path.abspath(__file__), "--worker", in_path, out_path],
            check=True,
            env=env,
        )
        return np.load(out_path)


if __name__ == "__main__":
    if len(sys.argv) >= 4 and sys.argv[1] == "--worker":
        data = dict(np.load(sys.argv[2]))
        out, _ = _run(data)
        np.save(sys.argv[3], out)
